# revision 13
# baseline (speedup 1.0000x reference)
"""Trainium2 Bass kernel for per-expert 2-layer MLP (grouped GEMM -> GELU -> grouped GEMM).

reference: hidden = einsum('end,edh->enh', x, w1); gelu(erf); out = einsum('enh,ehd->end', h, w2)
shapes:    x [16, 2048, 1024] f32, w1 [16, 1024, 4096] f32, w2 [16, 4096, 1024] f32

Expert-parallel over 8 NeuronCores: core c owns experts [2c, 2c+1], no
cross-core communication.  Per core, per expert:

  phase A:  actT[h, n] = gelu(w1[d, h].T @ xT[d, n])   (PE matmul, contraction d)
  phase B:  out[n, d'] = actT[h, n].T @ w2[h, d']      (PE matmul, contraction h)

Layout trick: matmul1 with w1 as the stationary operand directly yields
hidden TRANSPOSED ([h, n]) which is exactly the lhsT layout matmul2 needs.
x is pre-transposed (and pre-cast to bf16, like the weights) on the host as
part of sharding, so every device-side DMA is a natural contiguous load and
the PE does nothing but the 4096 productive matmuls.  Matmuls run in bf16
with fp32 PSUM accumulation; GELU (erf) runs on ScalarE out of PSUM.
"""

import os
import sys

import numpy as np

for _p in ("/opt/trn_rl_repo", "/root/.axon_site/_ro/trn_rl_repo"):
    if os.path.isdir(_p) and _p not in sys.path:
        sys.path.append(_p)

import ml_dtypes

import concourse.bacc as bacc
import concourse.tile as tile
from concourse import mybir
from concourse.bass_utils import run_bass_kernel_spmd

E, N, D, H = 16, 2048, 1024, 4096
NCORES = 8
EPC = E // NCORES        # experts per core                     = 2
P = 128                  # SBUF partitions
FD = 512                 # matmul moving free dim
NB = 512                 # token block per phase-A/B iteration
N_BLOCKS = N // NB       # = 4
N_SUB = NB // P          # row sub-blocks per token block       = 4
KD = D // P              # d-blocks (contraction of matmul 1)   = 8
KH = H // P              # h-blocks (contraction of matmul 2)   = 32
DC = D // FD             # d' chunks (free dim of matmul 2)     = 2
BF16 = mybir.dt.bfloat16
F32 = mybir.dt.float32

_CACHE = {}


def _build():
    nc = bacc.Bacc(None, target_bir_lowering=False)
    xt_d = nc.declare_dram_parameter("xt", [EPC, D, N], BF16, isOutput=False)
    w1_d = nc.declare_dram_parameter("w1", [EPC, D, H], BF16, isOutput=False)
    w2_d = nc.declare_dram_parameter("w2", [EPC, H, D], BF16, isOutput=False)
    out_d = nc.declare_dram_parameter("out", [EPC, N, D], F32, isOutput=True)

    with (
        tile.TileContext(nc) as tc,
        tc.tile_pool(name="w1sb", bufs=1) as w1_pool,
        tc.tile_pool(name="w2sb", bufs=1) as w2_pool,
        tc.tile_pool(name="xT", bufs=2) as xt_pool,
        tc.tile_pool(name="actT", bufs=1) as act_pool,
        tc.tile_pool(name="osb", bufs=3) as out_pool,
        tc.tile_pool(name="ps_1", bufs=4, space="PSUM") as ps1_pool,
        tc.tile_pool(name="ps_2", bufs=4, space="PSUM") as ps2_pool,
    ):

        def emit_w1_loads(e):
            """4 batched strided DMAs, column-chunk-major: phase A's first
            h-blocks unblock after one 2MB chunk, and few triggers keep the
            HWDGE queue free (each dma_start costs ~0.6us of queue time)."""
            w1_sb = w1_pool.tile([P, KD, H], BF16, tag="w1")
            w1_view = w1_d[e].rearrange("(k p) h -> p k h", p=P)
            # tiny first slice so phase A's first h-block unblocks ASAP
            bounds = [0, P, 1024, 2048, 3072, H]
            for lo, hi in zip(bounds, bounds[1:]):
                nc.scalar.dma_start(
                    out=w1_sb[:, :, lo:hi], in_=w1_view[:, :, lo:hi]
                )
            return w1_sb

        def emit_w2_loads(e):
            w2_sb = w2_pool.tile([P, KH, D], BF16, tag="w2")
            w2_view = w2_d[e].rearrange("(h p) d -> p h d", p=P)
            HB = KH // 4
            for c in range(4):
                nc.scalar.dma_start(
                    out=w2_sb[:, c * HB : (c + 1) * HB, :],
                    in_=w2_view[:, c * HB : (c + 1) * HB, :],
                )
            return w2_sb

        def emit_x_loads(e, nb):
            n0 = nb * NB
            xt_sb = xt_pool.tile([P, KD, NB], BF16, tag="xT")
            xt_view = xt_d[e].rearrange("(k p) n -> p k n", p=P)
            nc.sync.dma_start(out=xt_sb[:, :, :], in_=xt_view[:, :, n0 : n0 + NB])
            return xt_sb

        def emit_phase_a(w1_sb, xt_sb):
            actT = act_pool.tile([P, KH, NB], BF16, tag="actT")
            for h in range(KH):
                ps1 = ps1_pool.tile([P, NB], F32, tag="ps1")
                for k in range(KD):
                    nc.tensor.matmul(
                        ps1,
                        lhsT=w1_sb[:, k, h * P : (h + 1) * P],
                        rhs=xt_sb[:, k, :],
                        start=(k == 0),
                        stop=(k == KD - 1),
                    )
                nc.scalar.activation(actT[:, h, :], ps1, mybir.ActivationFunctionType.Gelu)
            return actT

        def emit_phase_b(e, nb, actT, w2_sb):
            n0 = nb * NB
            for s in range(N_SUB):
                osb = out_pool.tile([P, D], F32, tag="osb")
                for c in range(DC):
                    ps2 = ps2_pool.tile([P, FD], F32, tag="ps2")
                    for h in range(KH):
                        nc.tensor.matmul(
                            ps2,
                            lhsT=actT[:, h, s * P : (s + 1) * P],
                            rhs=w2_sb[:, h, c * FD : (c + 1) * FD],
                            start=(h == 0),
                            stop=(h == KH - 1),
                        )
                    nc.vector.tensor_copy(osb[:, c * FD : (c + 1) * FD], ps2)
                nc.sync.dma_start(out=out_d[e, n0 + s * P : n0 + (s + 1) * P, :], in_=osb)

        w1_cur = emit_w1_loads(0)
        w1_next = None
        w2_cur = None
        for e in range(EPC):
            for nb in range(N_BLOCKS):
                xt_sb = emit_x_loads(e, nb)
                actT = emit_phase_a(w1_cur, xt_sb)
                if nb == 0:
                    w2_cur = emit_w2_loads(e)
                if nb == N_BLOCKS - 1 and e + 1 < EPC:
                    w1_next = emit_w1_loads(e + 1)
                emit_phase_b(e, nb, actT, w2_cur)
            w1_cur = w1_next

    nc.compile()
    return nc


def _get_nc():
    if "nc" not in _CACHE:
        _CACHE["nc"] = _build()
    return _CACHE["nc"]


def _run(inputs, trace=False):
    x = np.asarray(inputs["x"], dtype=np.float32).astype(ml_dtypes.bfloat16)
    w1 = np.asarray(inputs["w1"], dtype=np.float32).astype(ml_dtypes.bfloat16)
    w2 = np.asarray(inputs["w2"], dtype=np.float32).astype(ml_dtypes.bfloat16)
    xt = np.ascontiguousarray(np.swapaxes(x, 1, 2))  # [E, D, N]
    nc = _get_nc()
    in_maps = [
        {
            "xt": xt[c * EPC : (c + 1) * EPC],
            "w1": np.ascontiguousarray(w1[c * EPC : (c + 1) * EPC]),
            "w2": np.ascontiguousarray(w2[c * EPC : (c + 1) * EPC]),
        }
        for c in range(NCORES)
    ]
    res = run_bass_kernel_spmd(nc, in_maps, list(range(NCORES)), trace=trace)
    out = np.concatenate([res.results[c]["out"] for c in range(NCORES)], axis=0)
    return out.astype(np.float32, copy=False), res


def kernel(**inputs) -> np.ndarray:
    out, _ = _run(inputs, trace=False)
    return out


# revision 14
# speedup vs baseline: 1.0188x; 1.0188x over previous
"""Trainium2 Bass kernel for per-expert 2-layer MLP (grouped GEMM -> GELU -> grouped GEMM).

reference: hidden = einsum('end,edh->enh', x, w1); gelu(erf); out = einsum('enh,ehd->end', h, w2)
shapes:    x [16, 2048, 1024] f32, w1 [16, 1024, 4096] f32, w2 [16, 4096, 1024] f32

Expert-parallel over 8 NeuronCores: core c owns experts [2c, 2c+1], no
cross-core communication.  Per core, per expert:

  phase A:  actT[h, n] = gelu(w1[d, h].T @ xT[d, n])   (PE matmul, contraction d)
  phase B:  out[n, d'] = actT[h, n].T @ w2[h, d']      (PE matmul, contraction h)

Layout trick: matmul1 with w1 as the stationary operand directly yields
hidden TRANSPOSED ([h, n]) which is exactly the lhsT layout matmul2 needs.
x is pre-transposed (and pre-cast to fp16, like the weights) on the host as
part of sharding, so every device-side DMA is a natural contiguous load and
the PE does nothing but the 4096 productive matmuls.  Matmuls run in fp16
with fp32 PSUM accumulation; GELU (erf) runs on ScalarE out of PSUM.
"""

import os
import sys

import numpy as np

for _p in ("/opt/trn_rl_repo", "/root/.axon_site/_ro/trn_rl_repo"):
    if os.path.isdir(_p) and _p not in sys.path:
        sys.path.append(_p)

import concourse.bacc as bacc
import concourse.tile as tile
from concourse import mybir
from concourse.bass_utils import run_bass_kernel_spmd

E, N, D, H = 16, 2048, 1024, 4096
NCORES = 8
EPC = E // NCORES        # experts per core                     = 2
P = 128                  # SBUF partitions
FD = 512                 # matmul moving free dim
NB = 512                 # token block per phase-A/B iteration
N_BLOCKS = N // NB       # = 4
N_SUB = NB // P          # row sub-blocks per token block       = 4
KD = D // P              # d-blocks (contraction of matmul 1)   = 8
KH = H // P              # h-blocks (contraction of matmul 2)   = 32
DC = D // FD             # d' chunks (free dim of matmul 2)     = 2
F16 = mybir.dt.float16
F32 = mybir.dt.float32

_CACHE = {}


def _build():
    nc = bacc.Bacc(None, target_bir_lowering=False)
    xt_d = nc.declare_dram_parameter("xt", [EPC, D, N], F16, isOutput=False)
    w1_d = nc.declare_dram_parameter("w1", [EPC, D, H], F16, isOutput=False)
    w2_d = nc.declare_dram_parameter("w2", [EPC, H, D], F16, isOutput=False)
    out_d = nc.declare_dram_parameter("out", [EPC, N, D], F32, isOutput=True)

    with (
        tile.TileContext(nc) as tc,
        tc.tile_pool(name="w1sb", bufs=1) as w1_pool,
        tc.tile_pool(name="w2sb", bufs=1) as w2_pool,
        tc.tile_pool(name="xT", bufs=2) as xt_pool,
        tc.tile_pool(name="actT", bufs=1) as act_pool,
        tc.tile_pool(name="osb", bufs=3) as out_pool,
        tc.tile_pool(name="ps_1", bufs=4, space="PSUM") as ps1_pool,
        tc.tile_pool(name="ps_2", bufs=4, space="PSUM") as ps2_pool,
    ):

        def emit_w1_loads(e):
            """4 batched strided DMAs, column-chunk-major: phase A's first
            h-blocks unblock after one 2MB chunk, and few triggers keep the
            HWDGE queue free (each dma_start costs ~0.6us of queue time)."""
            w1_sb = w1_pool.tile([P, KD, H], F16, tag="w1")
            w1_view = w1_d[e].rearrange("(k p) h -> p k h", p=P)
            # tiny first slice so phase A's first h-block unblocks ASAP
            bounds = [0, P, 1024, 2048, 3072, H]
            for lo, hi in zip(bounds, bounds[1:]):
                nc.scalar.dma_start(
                    out=w1_sb[:, :, lo:hi], in_=w1_view[:, :, lo:hi]
                )
            return w1_sb

        def emit_w2_loads(e):
            w2_sb = w2_pool.tile([P, KH, D], F16, tag="w2")
            w2_view = w2_d[e].rearrange("(h p) d -> p h d", p=P)
            HB = KH // 4
            for c in range(4):
                nc.scalar.dma_start(
                    out=w2_sb[:, c * HB : (c + 1) * HB, :],
                    in_=w2_view[:, c * HB : (c + 1) * HB, :],
                )
            return w2_sb

        def emit_x_loads(e, nb):
            n0 = nb * NB
            xt_sb = xt_pool.tile([P, KD, NB], F16, tag="xT")
            xt_view = xt_d[e].rearrange("(k p) n -> p k n", p=P)
            nc.sync.dma_start(out=xt_sb[:, :, :], in_=xt_view[:, :, n0 : n0 + NB])
            return xt_sb

        def emit_phase_a(w1_sb, xt_sb):
            actT = act_pool.tile([P, KH, NB], F16, tag="actT")
            for h in range(KH):
                ps1 = ps1_pool.tile([P, NB], F32, tag="ps1")
                for k in range(KD):
                    nc.tensor.matmul(
                        ps1,
                        lhsT=w1_sb[:, k, h * P : (h + 1) * P],
                        rhs=xt_sb[:, k, :],
                        start=(k == 0),
                        stop=(k == KD - 1),
                    )
                nc.scalar.activation(actT[:, h, :], ps1, mybir.ActivationFunctionType.Gelu)
            return actT

        def emit_phase_b(e, nb, actT, w2_sb):
            n0 = nb * NB
            for s in range(N_SUB):
                osb = out_pool.tile([P, D], F32, tag="osb")
                for c in range(DC):
                    ps2 = ps2_pool.tile([P, FD], F32, tag="ps2")
                    for h in range(KH):
                        nc.tensor.matmul(
                            ps2,
                            lhsT=actT[:, h, s * P : (s + 1) * P],
                            rhs=w2_sb[:, h, c * FD : (c + 1) * FD],
                            start=(h == 0),
                            stop=(h == KH - 1),
                        )
                    nc.vector.tensor_copy(osb[:, c * FD : (c + 1) * FD], ps2)
                nc.sync.dma_start(out=out_d[e, n0 + s * P : n0 + (s + 1) * P, :], in_=osb)

        w1_cur = emit_w1_loads(0)
        w1_next = None
        w2_cur = None
        for e in range(EPC):
            for nb in range(N_BLOCKS):
                xt_sb = emit_x_loads(e, nb)
                actT = emit_phase_a(w1_cur, xt_sb)
                if nb == 0:
                    w2_cur = emit_w2_loads(e)
                if nb == N_BLOCKS - 1 and e + 1 < EPC:
                    w1_next = emit_w1_loads(e + 1)
                emit_phase_b(e, nb, actT, w2_cur)
            w1_cur = w1_next

    nc.compile()
    return nc


def _get_nc():
    if "nc" not in _CACHE:
        _CACHE["nc"] = _build()
    return _CACHE["nc"]


def _run(inputs, trace=False, trace_cores=None):
    x = np.asarray(inputs["x"], dtype=np.float32).astype(np.float16)
    w1 = np.asarray(inputs["w1"], dtype=np.float32).astype(np.float16)
    w2 = np.asarray(inputs["w2"], dtype=np.float32).astype(np.float16)
    xt = np.ascontiguousarray(np.swapaxes(x, 1, 2))  # [E, D, N]
    nc = _get_nc()
    in_maps = [
        {
            "xt": xt[c * EPC : (c + 1) * EPC],
            "w1": np.ascontiguousarray(w1[c * EPC : (c + 1) * EPC]),
            "w2": np.ascontiguousarray(w2[c * EPC : (c + 1) * EPC]),
        }
        for c in range(NCORES)
    ]
    res = run_bass_kernel_spmd(
        nc, in_maps, list(range(NCORES)), trace=trace, trace_cores=trace_cores
    )
    out = np.concatenate([res.results[c]["out"] for c in range(NCORES)], axis=0)
    return out.astype(np.float32, copy=False), res


def kernel(**inputs) -> np.ndarray:
    out, _ = _run(inputs, trace=False)
    return out
